# revision 4
# baseline (speedup 1.0000x reference)
"""CAL-GAT kernel for 8 Trainium2 NeuronCores.

Sharding: nodes split into 8 contiguous shards (batch-sorted); edges assigned
to the core owning their dst node; small weights replicated. Per-edge gathers
use indirect DMA from all-gathered node tables; segment sums are one-hot
matmuls accumulated in PSUM per 128-dst-node group (collision-free).
Final pooled classifier MLPs ([G,64] x small) run on host after summing the
per-core pool partials.
"""
import sys
sys.path.insert(0, '/opt/trn_rl_repo')
import numpy as np

import concourse.bacc as bacc
import concourse.bass as bass
import concourse.mybir as mybir
import concourse.tile as tile
from concourse.bass_utils import run_bass_kernel_spmd

F32 = mybir.dt.float32
I32 = mybir.dt.int32
AF = mybir.ActivationFunctionType
ALU = mybir.AluOpType
AX = mybir.AxisListType

NC = 8
HEADS, HID, HD, C, SLOPE = 8, 8, 64, 10, 0.2


def _wrap(a, pad_val):
    """[S] -> [128, S/128]; slot s -> (s%128, s//128)."""
    S = len(a)
    assert S % 128 == 0
    return np.ascontiguousarray(a.reshape(S // 128, 128).T)


def _edge_lists(src, dst, SH, NG, n_cores):
    """Per-core dst-grouped padded slot arrays. Returns (T[g] shared, per-core
    dict of slot arrays)."""
    core = dst // SH
    g = (dst % SH) // 128
    cnts = np.zeros((n_cores, NG), np.int64)
    for c in range(n_cores):
        m = core == c
        cnts[c] = np.bincount(g[m], minlength=NG)
    T = np.maximum(1, -(-cnts.max(axis=0) // 128))          # [NG]
    goff = np.concatenate([[0], np.cumsum(T * 128)])        # slot offset per group
    S = int(goff[-1])
    per_core = []
    for c in range(n_cores):
        m = core == c
        sc, dc, gc = src[m], dst[m] % SH, g[m]
        order = np.argsort(gc, kind='stable')
        sc, dc, gc = sc[order], dc[order], gc[order]
        # position within group
        gstart = np.concatenate([[0], np.cumsum(np.bincount(gc, minlength=NG))])
        pos = np.arange(len(gc)) - gstart[gc]
        slot = goff[gc] + pos
        a_src = np.zeros(S, np.int32)
        a_dst = np.zeros(S, np.int32)
        a_rel = np.full(S, -1.0, np.float32)
        a_src[slot] = sc
        a_dst[slot] = dc
        a_rel[slot] = (dc - gc * 128).astype(np.float32)
        per_core.append(dict(src=a_src, dst=a_dst, rel=a_rel))
    return T.astype(int), S, per_core


def kernel(**inputs):
    inp = {k: np.asarray(v) for k, v in inputs.items()}
    x = inp['x'].astype(np.float32)
    ei = inp['edge_index'].astype(np.int64)
    batch = inp['batch'].astype(np.int64)
    N, F_IN = x.shape
    E = ei.shape[1]
    G = int(batch.max()) + 1 if len(batch) else 1
    G = max(G, 1)
    GP = 128  # padded graph count for pool matmul
    assert N % NC == 0
    SH = N // NC
    NG = -(-SH // 128)
    LASTR = SH - (NG - 1) * 128          # rows in last group

    W = {k: inp[k].astype(np.float32) for k in inp if k not in ('x', 'edge_index', 'batch')}

    src, dst = ei[0], ei[1]
    loop = np.arange(N, dtype=np.int64)
    gat_T, gat_S, gat_pc = _edge_lists(np.concatenate([src, loop]),
                                       np.concatenate([dst, loop]), SH, NG, NC)
    ea_T, ea_S, ea_pc = _edge_lists(src, dst, SH, NG, NC)

    def blockdiag(a):  # [HEADS, HID] -> [HD, HEADS]
        m = np.zeros((HD, HEADS), np.float32)
        for h in range(HEADS):
            m[h * HID:(h + 1) * HID, h] = a[h]
        return m

    iota_row = np.tile(np.arange(128, dtype=np.float32), (128, 1))
    ident = np.eye(128, dtype=np.float32)

    # per-core host inputs
    in_maps = []
    for c in range(NC):
        b_loc = batch[c * SH:(c + 1) * SH].astype(np.float32)
        bw = np.full((128, NG), -1.0, np.float32)
        bw_flat = np.full(NG * 128, -1.0, np.float32)
        bw_flat[:SH] = b_loc
        bw = np.ascontiguousarray(bw_flat.reshape(NG, 128).T)
        m = dict(
            xT=np.ascontiguousarray(x[c * SH:(c + 1) * SH].T),
            gat_src=_wrap(gat_pc[c]['src'], 0), gat_dst=_wrap(gat_pc[c]['dst'], 0),
            gat_rel=_wrap(gat_pc[c]['rel'], -1),
            ea_src=_wrap(ea_pc[c]['src'], 0), ea_dst=_wrap(ea_pc[c]['dst'], 0),
            ea_rel=_wrap(ea_pc[c]['rel'], -1),
            batchw=bw, iota=iota_row, ident=ident,
            W1=W['W1'], AS1=blockdiag(W['as1']), AD1=blockdiag(W['ad1']),
            W2=W['W2'], AS2=blockdiag(W['as2']), AD2=blockdiag(W['ad2']),
            NAW1=W['na_w1'],
            NAW2B=np.tile(W['na_w2'].T.reshape(-1), (128, 1)),   # [128, 2*32]
            WA=W['ea_w1'][:HD], WB=W['ea_w1'][HD:],
            EAW2B=np.tile(W['ea_w2'].T.reshape(-1), (128, 1)),   # [128, 2*64]
            GCW=W['gc_w'], GTW=W['gt_w'], RCW=W['rc_w'], RTW=W['rt_w'],
            B1=np.tile(W['b1'], (128, 1)), B2=np.tile(W['b2'], (128, 1)),
            NAB1=np.tile(W['na_b1'], (128, 1)), NAB2=np.tile(W['na_b2'], (128, 1)),
            EAB1=np.tile(W['ea_b1'], (128, 1)), EAB2=np.tile(W['ea_b2'], (128, 1)),
            GCB=np.tile(W['gc_b'], (128, 1)), GTB=np.tile(W['gt_b'], (128, 1)),
            RCB=np.tile(W['rc_b'], (128, 1)), RTB=np.tile(W['rt_b'], (128, 1)),
        )
        in_maps.append(m)

    nc_prog = _build(N, F_IN, SH, NG, LASTR, gat_T, gat_S, ea_T, ea_S)
    res = run_bass_kernel_spmd(nc_prog, in_maps, core_ids=list(range(NC)))

    pool = np.zeros((128, 128), np.float64)
    for c in range(NC):
        pool += res.results[c]['pool'].astype(np.float64)
    hGc = pool[:G, :HD].astype(np.float32)
    hGt = pool[:G, HD:].astype(np.float32)

    def mlp2(h, w1, b1, w2, b2):
        return np.maximum(h @ w1 + b1, 0.0) @ w2 + b2

    zc = mlp2(hGc, W['cc_w1'], W['cc_b1'], W['cc_w2'], W['cc_b2'])
    zt = mlp2(hGt, W['ct_w1'], W['ct_b1'], W['ct_w2'], W['ct_b2'])
    zp = mlp2(np.concatenate([hGc, hGt], 1), W['cm_w1'], W['cm_b1'], W['cm_w2'], W['cm_b2'])
    return (zc.astype(np.float32), zt.astype(np.float32), zp.astype(np.float32), hGc, hGt)


def _build(N, F_IN, SH, NG, LASTR, gat_T, gat_S, ea_T, ea_S):
    nc = bacc.Bacc("TRN2", target_bir_lowering=False, debug=False)
    SHP = NG * 128                    # padded local rows

    def ein(name, shape, dt=F32):
        return nc.dram_tensor(name, shape, dt, kind="ExternalInput")

    xT = ein("xT", [F_IN, SH])
    gat_src = ein("gat_src", [128, gat_S // 128], I32)
    gat_dst = ein("gat_dst", [128, gat_S // 128], I32)
    gat_rel = ein("gat_rel", [128, gat_S // 128])
    ea_src = ein("ea_src", [128, ea_S // 128], I32)
    ea_dst = ein("ea_dst", [128, ea_S // 128], I32)
    ea_rel = ein("ea_rel", [128, ea_S // 128])
    batchw = ein("batchw", [128, NG])
    iota_in = ein("iota", [128, 128]); ident_in = ein("ident", [128, 128])
    W1 = ein("W1", [F_IN, HD]); AS1 = ein("AS1", [HD, HEADS]); AD1 = ein("AD1", [HD, HEADS])
    W2 = ein("W2", [HD, HD]); AS2 = ein("AS2", [HD, HEADS]); AD2 = ein("AD2", [HD, HEADS])
    NAW1 = ein("NAW1", [HD, 32]); NAW2B = ein("NAW2B", [128, 64])
    WA = ein("WA", [HD, HD]); WB = ein("WB", [HD, HD]); EAW2B = ein("EAW2B", [128, 128])
    GCW = ein("GCW", [HD, HD]); GTW = ein("GTW", [HD, HD])
    RCW = ein("RCW", [HD, HD]); RTW = ein("RTW", [HD, HD])
    B1 = ein("B1", [128, HD]); B2 = ein("B2", [128, HD])
    NAB1 = ein("NAB1", [128, 32]); NAB2 = ein("NAB2", [128, 2])
    EAB1 = ein("EAB1", [128, HD]); EAB2 = ein("EAB2", [128, 2])
    GCB = ein("GCB", [128, HD]); GTB = ein("GTB", [128, HD])
    RCB = ein("RCB", [128, HD]); RTB = ein("RTB", [128, HD])

    pool_out = nc.dram_tensor("pool", [128, 128], F32, kind="ExternalOutput")

    # internal DRAM
    T1loc = nc.dram_tensor("T1loc", [SH, 72], F32)
    T1full = nc.dram_tensor("T1full", [N, 72], F32)
    T2loc = nc.dram_tensor("T2loc", [SH, 72], F32)
    T2full = nc.dram_tensor("T2full", [N, 72], F32)
    Thaloc = nc.dram_tensor("Thaloc", [SH, HD], F32)
    Thafull = nc.dram_tensor("Thafull", [N, HD], F32)
    T4loc = nc.dram_tensor("T4loc", [SH, 128], F32)
    T4full = nc.dram_tensor("T4full", [N, 128], F32)
    ed1tab = nc.dram_tensor("ed1tab", [SHP, 8], F32)
    ed2tab = nc.dram_tensor("ed2tab", [SHP, 8], F32)
    hbtab = nc.dram_tensor("hbtab", [SHP, HD], F32)
    hcptab = nc.dram_tensor("hcptab", [SHP, HD], F32)
    hloctab = nc.dram_tensor("hloctab", [SHP, HD], F32)
    htptab = nc.dram_tensor("htptab", [SHP, HD], F32)
    betatab = nc.dram_tensor("betatab", [128, ea_S // 128, 2], F32)

    RG = list(range(8))

    with tile.TileContext(nc) as tc:
        import contextlib
        ctx = contextlib.ExitStack()
        with ctx:
            res_p = ctx.enter_context(tc.tile_pool(name="res", bufs=1))
            wrk = ctx.enter_context(tc.tile_pool(name="wrk", bufs=2))
            ps = ctx.enter_context(tc.tile_pool(name="ps", bufs=2, space="PSUM"))
            acc_p = ctx.enter_context(tc.tile_pool(name="acc", bufs=1, space="PSUM"))

            def load_const(src_t, shape, dt=F32):
                t = res_p.tile(shape, dt, tag=src_t.name + "_s")
                nc.sync.dma_start(t[:], src_t[:])
                return t

            iota = load_const(iota_in, [128, 128])
            ident = load_const(ident_in, [128, 128])
            gsrc = load_const(gat_src, [128, gat_S // 128], I32)
            gdst = load_const(gat_dst, [128, gat_S // 128], I32)
            grel = load_const(gat_rel, [128, gat_S // 128])
            esrc = load_const(ea_src, [128, ea_S // 128], I32)
            edst = load_const(ea_dst, [128, ea_S // 128], I32)
            erel = load_const(ea_rel, [128, ea_S // 128])
            bw_s = load_const(batchw, [128, NG])
            w1_s = load_const(W1, [F_IN, HD]); as1_s = load_const(AS1, [HD, 8]); ad1_s = load_const(AD1, [HD, 8])
            w2_s = load_const(W2, [HD, HD]); as2_s = load_const(AS2, [HD, 8]); ad2_s = load_const(AD2, [HD, 8])
            naw1_s = load_const(NAW1, [HD, 32]); naw2_s = load_const(NAW2B, [128, 64])
            wa_s = load_const(WA, [HD, HD]); wb_s = load_const(WB, [HD, HD]); eaw2_s = load_const(EAW2B, [128, 128])
            gcw_s = load_const(GCW, [HD, HD]); gtw_s = load_const(GTW, [HD, HD])
            rcw_s = load_const(RCW, [HD, HD]); rtw_s = load_const(RTW, [HD, HD])
            b1_s = load_const(B1, [128, HD]); b2_s = load_const(B2, [128, HD])
            nab1_s = load_const(NAB1, [128, 32]); nab2_s = load_const(NAB2, [128, 2])
            eab1_s = load_const(EAB1, [128, HD]); eab2_s = load_const(EAB2, [128, 2])
            gcb_s = load_const(GCB, [128, HD]); gtb_s = load_const(GTB, [128, HD])
            rcb_s = load_const(RCB, [128, HD]); rtb_s = load_const(RTB, [128, HD])

            hT = res_p.tile([HD, SHP], F32, tag="hT")        # H1^T then H^T
            alpha = res_p.tile([128, NG, 2], F32, tag="alpha")
            disr = res_p.tile([128, NG, 2], F32, tag="disr")

            def nrows(g):
                return 128 if g < NG - 1 else LASTR

            # ---------------- node phase 1: T1 = [x@W1 | es1], ed1 ----------------
            for g in range(NG):
                nr = nrows(g)
                xt = wrk.tile([F_IN, 128], F32, tag="xt")
                nc.sync.dma_start(xt[:, :nr], xT[:, g * 128:g * 128 + nr])
                h1p = ps.tile([128, HD], F32, tag="mmA")
                nc.tensor.matmul(h1p[:nr], lhsT=xt[:, :nr], rhs=w1_s[:], start=True, stop=True)
                h1tp = ps.tile([HD, 128], F32, tag="mmB")
                nc.tensor.matmul(h1tp[:, :nr], lhsT=w1_s[:], rhs=xt[:, :nr], start=True, stop=True)
                h1t = wrk.tile([HD, 128], F32, tag="h1t")
                nc.vector.tensor_copy(h1t[:, :nr], h1tp[:, :nr])
                esp = ps.tile([128, 8], F32, tag="mmA")
                nc.tensor.matmul(esp[:nr], lhsT=h1t[:, :nr], rhs=as1_s[:], start=True, stop=True)
                edp = ps.tile([128, 8], F32, tag="mmB")
                nc.tensor.matmul(edp[:nr], lhsT=h1t[:, :nr], rhs=ad1_s[:], start=True, stop=True)
                row = wrk.tile([128, 72], F32, tag="row72")
                nc.vector.tensor_copy(row[:nr, :HD], h1p[:nr])
                nc.vector.tensor_copy(row[:nr, HD:72], esp[:nr])
                nc.sync.dma_start(T1loc[g * 128:g * 128 + nr, :], row[:nr])
                edr = wrk.tile([128, 8], F32, tag="edr")
                nc.vector.tensor_copy(edr[:nr], edp[:nr])
                nc.sync.dma_start(ed1tab[g * 128:g * 128 + nr, :], edr[:nr])

            nc.gpsimd.collective_compute("AllGather", ALU.bypass, ins=[T1loc.ap().opt()],
                                         outs=[T1full.ap().opt()], replica_groups=[RG])

            # ---------------- GAT edge pass helper ----------------
            def gat_pass(Tfull, edtab, idx_s, idx_d, idx_r, T_list, bias_s, do_elu, fin_cb):
                col0 = 0
                for g in range(NG):
                    T = int(T_list[g])
                    nr = nrows(g)
                    hs = wrk.tile([128, 13, 72], F32, tag="hs")
                    ed = wrk.tile([128, 13, 8], F32, tag="ed")
                    oh = wrk.tile([128, 13 * 128], F32, tag="oh")
                    v72 = wrk.tile([128, 13, 72], F32, tag="v72")
                    for j in range(T):
                        nc.gpsimd.indirect_dma_start(
                            out=hs[:, j], out_offset=None, in_=Tfull[:, :],
                            in_offset=bass.IndirectOffsetOnAxis(ap=idx_s[:, col0 + j:col0 + j + 1], axis=0))
                        nc.gpsimd.indirect_dma_start(
                            out=ed[:, j], out_offset=None, in_=edtab[:, :],
                            in_offset=bass.IndirectOffsetOnAxis(ap=idx_d[:, col0 + j:col0 + j + 1], axis=0))
                    # one-hot [128e, T*128n]
                    nc.vector.tensor_tensor(
                        out=oh[:, :T * 128],
                        in0=iota[:, None, :].to_broadcast([128, T, 128]),
                        in1=idx_r[:, col0:col0 + T][:, :, None].to_broadcast([128, T, 128]),
                        op=ALU.is_equal)
                    logit = wrk.tile([128, 13, 8], F32, tag="logit")
                    nc.vector.tensor_tensor(out=logit[:, :T], in0=hs[:, :T, HD:72], in1=ed[:, :T], op=ALU.add)
                    lr2 = wrk.tile([128, 13, 8], F32, tag="lr2")
                    nc.vector.tensor_scalar_mul(lr2[:, :T], logit[:, :T], SLOPE)
                    nc.vector.tensor_tensor(out=logit[:, :T], in0=logit[:, :T], in1=lr2[:, :T], op=ALU.max)
                    ex = wrk.tile([128, 13, 8], F32, tag="ex")
                    nc.scalar.activation(ex[:, :T], logit[:, :T], AF.Exp)
                    nc.vector.tensor_tensor(
                        out=v72[:, :T, :HD].rearrange("p t (h d) -> p t h d", h=8),
                        in0=hs[:, :T, :HD].rearrange("p t (h d) -> p t h d", h=8),
                        in1=ex[:, :T, :, None].to_broadcast([128, T, 8, 8]),
                        op=ALU.mult)
                    nc.vector.tensor_copy(v72[:, :T, HD:72], ex[:, :T])
                    pacc = ps.tile([128, 72], F32, tag="pscc")
                    for j in range(T):
                        nc.tensor.matmul(pacc[:], lhsT=oh[:, j * 128:(j + 1) * 128], rhs=v72[:, j],
                                         start=(j == 0), stop=(j == T - 1))
                    # finalize: out = num/den + bias (+elu)
                    rec = wrk.tile([128, 8], F32, tag="rec")
                    nc.vector.reciprocal(rec[:nr], pacc[:nr, HD:72])
                    o64 = wrk.tile([128, HD], F32, tag="o64")
                    if nr < 128:
                        nc.vector.memset(o64[nr:], 0.0)
                    nc.vector.tensor_tensor(
                        out=o64[:nr].rearrange("p (h d) -> p h d", h=8),
                        in0=pacc[:nr, :HD].rearrange("p (h d) -> p h d", h=8),
                        in1=rec[:nr, :, None].to_broadcast([nr, 8, 8]),
                        op=ALU.mult)
                    nc.vector.tensor_tensor(out=o64[:nr], in0=o64[:nr], in1=bias_s[:nr], op=ALU.add)
                    if do_elu:
                        ng_t = wrk.tile([128, HD], F32, tag="ngt")
                        nc.vector.tensor_scalar_min(ng_t[:nr], o64[:nr], 0.0)
                        ex2 = wrk.tile([128, HD], F32, tag="ex2")
                        nc.scalar.activation(ex2[:nr], ng_t[:nr], AF.Exp)
                        nc.vector.tensor_scalar_add(ex2[:nr], ex2[:nr], -1.0)
                        nc.vector.tensor_scalar_max(o64[:nr], o64[:nr], 0.0)
                        nc.vector.tensor_tensor(out=o64[:nr], in0=o64[:nr], in1=ex2[:nr], op=ALU.add)
                    fin_cb(g, nr, o64)
                    col0 += T

            # ---------------- GAT1 edges -> H1 (elu), keep H1^T ----------------
            def fin1(g, nr, o64):
                tp = ps.tile([128, 128], F32, tag="mmB")
                nc.tensor.transpose(tp[:HD, :], o64[:], ident[:])
                nc.vector.tensor_copy(hT[:, g * 128:(g + 1) * 128], tp[:HD, :])

            gat_pass(T1full, ed1tab, gsrc, gdst, grel, gat_T, b1_s, True, fin1)

            # ---------------- node phase 2: T2 = [H1@W2 | es2], ed2 ----------------
            for g in range(NG):
                nr = nrows(g)
                h1t_g = hT[:, g * 128:g * 128 + 128]
                h2p = ps.tile([128, HD], F32, tag="mmA")
                nc.tensor.matmul(h2p[:], lhsT=h1t_g, rhs=w2_s[:], start=True, stop=True)
                h2tp = ps.tile([HD, 128], F32, tag="mmB")
                nc.tensor.matmul(h2tp[:], lhsT=w2_s[:], rhs=h1t_g, start=True, stop=True)
                h2t = wrk.tile([HD, 128], F32, tag="h1t")
                nc.vector.tensor_copy(h2t[:], h2tp[:])
                esp = ps.tile([128, 8], F32, tag="mmA")
                nc.tensor.matmul(esp[:], lhsT=h2t[:], rhs=as2_s[:], start=True, stop=True)
                edp = ps.tile([128, 8], F32, tag="mmB")
                nc.tensor.matmul(edp[:], lhsT=h2t[:], rhs=ad2_s[:], start=True, stop=True)
                row = wrk.tile([128, 72], F32, tag="row72")
                nc.vector.tensor_copy(row[:, :HD], h2p[:])
                nc.vector.tensor_copy(row[:, HD:72], esp[:])
                nc.sync.dma_start(T2loc[g * 128:g * 128 + nr, :], row[:nr])
                edr = wrk.tile([128, 8], F32, tag="edr")
                nc.vector.tensor_copy(edr[:], edp[:])
                nc.sync.dma_start(ed2tab[g * 128:g * 128 + nr, :], edr[:nr])

            nc.gpsimd.collective_compute("AllGather", ALU.bypass, ins=[T2loc.ap().opt()],
                                         outs=[T2full.ap().opt()], replica_groups=[RG])

            # ---------------- GAT2 edges -> H (no act), keep H^T + H node-major ----------------
            def fin2(g, nr, o64):
                nc.sync.dma_start(hloctab[g * 128:(g + 1) * 128, :], o64[:])
                tp = ps.tile([128, 128], F32, tag="mmB")
                nc.tensor.transpose(tp[:HD, :], o64[:], ident[:])
                nc.vector.tensor_copy(hT[:, g * 128:(g + 1) * 128], tp[:HD, :])

            gat_pass(T2full, ed2tab, gsrc, gdst, grel, gat_T, b2_s, False, fin2)

            # ---------------- node phase 3a: ha, hb, alpha, hc', ht' ----------------
            for g in range(NG):
                nr = nrows(g)
                ht_g = hT[:, g * 128:g * 128 + 128]
                hap = ps.tile([128, HD], F32, tag="mmA")
                nc.tensor.matmul(hap[:], lhsT=ht_g, rhs=wa_s[:], start=True, stop=True)
                har = wrk.tile([128, HD], F32, tag="har")
                nc.vector.tensor_tensor(out=har[:], in0=hap[:], in1=eab1_s[:], op=ALU.add)
                nc.sync.dma_start(Thaloc[g * 128:g * 128 + nr, :], har[:nr])
                hbp = ps.tile([128, HD], F32, tag="mmA")
                nc.tensor.matmul(hbp[:], lhsT=ht_g, rhs=wb_s[:], start=True, stop=True)
                hbr = wrk.tile([128, HD], F32, tag="hbr")
                nc.vector.tensor_copy(hbr[:], hbp[:])
                nc.sync.dma_start(hbtab[g * 128:(g + 1) * 128, :], hbr[:])
                # node attention MLP
                nhp = ps.tile([128, 32], F32, tag="mmA")
                nc.tensor.matmul(nhp[:], lhsT=ht_g, rhs=naw1_s[:], start=True, stop=True)
                nh = wrk.tile([128, 32], F32, tag="nh")
                nc.vector.tensor_tensor(out=nh[:], in0=nhp[:], in1=nab1_s[:], op=ALU.add)
                nc.scalar.activation(nh[:], nh[:], AF.Relu)
                z = wrk.tile([128, 2], F32, tag="z2")
                tmp = wrk.tile([128, 32], F32, tag="tmp32")
                nc.vector.tensor_tensor(out=tmp[:], in0=nh[:], in1=naw2_s[:, :32], op=ALU.mult)
                nc.vector.reduce_sum(z[:, 0:1], tmp[:], axis=AX.X)
                nc.vector.tensor_tensor(out=tmp[:], in0=nh[:], in1=naw2_s[:, 32:64], op=ALU.mult)
                nc.vector.reduce_sum(z[:, 1:2], tmp[:], axis=AX.X)
                nc.vector.tensor_tensor(out=z[:], in0=z[:], in1=nab2_s[:], op=ALU.add)
                nc.scalar.activation(z[:], z[:], AF.Exp)
                zs = wrk.tile([128, 1], F32, tag="zs")
                nc.vector.reduce_sum(zs[:], z[:], axis=AX.X)
                nc.vector.reciprocal(zs[:], zs[:])
                nc.vector.tensor_scalar(out=alpha[:, g, :], in0=z[:], scalar1=zs[:],
                                        scalar2=None, op0=ALU.mult)
                # hc' = (H*ac)@gc_w ; ht'
                hl = wrk.tile([128, HD], F32, tag="hll")
                nc.sync.dma_start(hl[:], hloctab[g * 128:(g + 1) * 128, :])
                for br, (gw, tab) in enumerate(((gcw_s, hcptab), (gtw_s, htptab))):
                    dia = wrk.tile([128, 128], F32, tag="dia")
                    nc.vector.tensor_scalar(out=dia[:], in0=ident[:], scalar1=alpha[:, g, br:br + 1],
                                            scalar2=None, op0=ALU.mult)
                    hctp = ps.tile([HD, 128], F32, tag="mmB")
                    nc.tensor.matmul(hctp[:], lhsT=hl[:], rhs=dia[:], start=True, stop=True)
                    hct = wrk.tile([HD, 128], F32, tag="hct")
                    nc.vector.tensor_copy(hct[:], hctp[:])
                    hpp = ps.tile([128, HD], F32, tag="mmA")
                    nc.tensor.matmul(hpp[:], lhsT=hct[:], rhs=gw[:], start=True, stop=True)
                    hpr = wrk.tile([128, HD], F32, tag="hpr")
                    nc.vector.tensor_copy(hpr[:], hpp[:])
                    nc.sync.dma_start(tab[g * 128:(g + 1) * 128, :], hpr[:])

            nc.gpsimd.collective_compute("AllGather", ALU.bypass, ins=[Thaloc.ap().opt()],
                                         outs=[Thafull.ap().opt()], replica_groups=[RG])

            # ---------------- EA edges: beta + deg -> dis ----------------
            col0 = 0
            for g in range(NG):
                T = int(ea_T[g])
                nr = nrows(g)
                ha = wrk.tile([128, 13, HD], F32, tag="hs")
                hb = wrk.tile([128, 13, HD], F32, tag="ed")
                oh = wrk.tile([128, 13 * 128], F32, tag="oh")
                for j in range(T):
                    nc.gpsimd.indirect_dma_start(
                        out=ha[:, j], out_offset=None, in_=Thafull[:, :],
                        in_offset=bass.IndirectOffsetOnAxis(ap=esrc[:, col0 + j:col0 + j + 1], axis=0))
                    nc.gpsimd.indirect_dma_start(
                        out=hb[:, j], out_offset=None, in_=hbtab[:, :],
                        in_offset=bass.IndirectOffsetOnAxis(ap=edst[:, col0 + j:col0 + j + 1], axis=0))
                nc.vector.tensor_tensor(
                    out=oh[:, :T * 128],
                    in0=iota[:, None, :].to_broadcast([128, T, 128]),
                    in1=erel[:, col0:col0 + T][:, :, None].to_broadcast([128, T, 128]),
                    op=ALU.is_equal)
                nc.vector.tensor_tensor(out=ha[:, :T], in0=ha[:, :T], in1=hb[:, :T], op=ALU.add)
                nc.scalar.activation(ha[:, :T], ha[:, :T], AF.Relu)
                z = wrk.tile([128, 13, 2], F32, tag="zea")
                tmp = wrk.tile([128, 13, HD], F32, tag="v72")
                nc.vector.tensor_tensor(out=tmp[:, :T], in0=ha[:, :T],
                                        in1=eaw2_s[:, None, :HD].to_broadcast([128, T, HD]), op=ALU.mult)
                nc.vector.reduce_sum(z[:, :T, 0:1], tmp[:, :T], axis=AX.X)
                nc.vector.tensor_tensor(out=tmp[:, :T], in0=ha[:, :T],
                                        in1=eaw2_s[:, None, HD:128].to_broadcast([128, T, HD]), op=ALU.mult)
                nc.vector.reduce_sum(z[:, :T, 1:2], tmp[:, :T], axis=AX.X)
                nc.vector.tensor_tensor(out=z[:, :T], in0=z[:, :T],
                                        in1=eab2_s[:, None, :].to_broadcast([128, T, 2]), op=ALU.add)
                nc.scalar.activation(z[:, :T], z[:, :T], AF.Exp)
                zs = wrk.tile([128, 13, 1], F32, tag="zsea")
                nc.vector.reduce_sum(zs[:, :T], z[:, :T], axis=AX.X)
                nc.vector.reciprocal(zs[:, :T], zs[:, :T])
                beta = wrk.tile([128, 13, 2], F32, tag="beta")
                nc.vector.tensor_tensor(out=beta[:, :T], in0=z[:, :T],
                                        in1=zs[:, :T].to_broadcast([128, T, 2]), op=ALU.mult)
                nc.sync.dma_start(betatab[:, col0:col0 + T, :], beta[:, :T])
                pd = ps.tile([128, 2], F32, tag="pscc")
                for j in range(T):
                    nc.tensor.matmul(pd[:], lhsT=oh[:, j * 128:(j + 1) * 128], rhs=beta[:, j],
                                     start=(j == 0), stop=(j == T - 1))
                dg = wrk.tile([128, 2], F32, tag="dg")
                nc.vector.tensor_scalar_add(dg[:], pd[:], 1.0)
                nc.scalar.activation(dg[:], dg[:], AF.Sqrt)
                nc.vector.reciprocal(disr[:, g, :], dg[:])
                col0 += T

            # ---------------- node 3b: T4 = [hc'*disc | ht'*dist] ----------------
            for g in range(NG):
                nr = nrows(g)
                t4 = wrk.tile([128, 128], F32, tag="t4r")
                for br, tab in enumerate((hcptab, htptab)):
                    hp = wrk.tile([128, HD], F32, tag="hpl")
                    nc.sync.dma_start(hp[:], tab[g * 128:(g + 1) * 128, :])
                    nc.vector.tensor_scalar(out=t4[:, br * HD:(br + 1) * HD], in0=hp[:],
                                            scalar1=disr[:, g, br:br + 1], scalar2=None, op0=ALU.mult)
                nc.sync.dma_start(T4loc[g * 128:g * 128 + nr, :], t4[:nr])

            nc.gpsimd.collective_compute("AllGather", ALU.bypass, ins=[T4loc.ap().opt()],
                                         outs=[T4full.ap().opt()], replica_groups=[RG])

            # ---------------- GCN edges + readout pool ----------------
            pp = acc_p.tile([128, 128], F32)
            col0 = 0
            for g in range(NG):
                T = int(ea_T[g])
                nr = nrows(g)
                t4s = wrk.tile([128, 13, 128], F32, tag="t4s")
                oh = wrk.tile([128, 13 * 128], F32, tag="oh")
                beta = wrk.tile([128, 13, 2], F32, tag="beta")
                nc.sync.dma_start(beta[:, :T], betatab[:, col0:col0 + T, :])
                for j in range(T):
                    nc.gpsimd.indirect_dma_start(
                        out=t4s[:, j], out_offset=None, in_=T4full[:, :],
                        in_offset=bass.IndirectOffsetOnAxis(ap=esrc[:, col0 + j:col0 + j + 1], axis=0))
                nc.vector.tensor_tensor(
                    out=oh[:, :T * 128],
                    in0=iota[:, None, :].to_broadcast([128, T, 128]),
                    in1=erel[:, col0:col0 + T][:, :, None].to_broadcast([128, T, 128]),
                    op=ALU.is_equal)
                v = wrk.tile([128, 13, 128], F32, tag="vg")
                nc.vector.tensor_tensor(out=v[:, :T, :HD], in0=t4s[:, :T, :HD],
                                        in1=beta[:, :T, 0:1].to_broadcast([128, T, HD]), op=ALU.mult)
                nc.vector.tensor_tensor(out=v[:, :T, HD:], in0=t4s[:, :T, HD:],
                                        in1=beta[:, :T, 1:2].to_broadcast([128, T, HD]), op=ALU.mult)
                pg = ps.tile([128, 128], F32, tag="pscc")
                for j in range(T):
                    nc.tensor.matmul(pg[:], lhsT=oh[:, j * 128:(j + 1) * 128], rhs=v[:, j],
                                     start=(j == 0), stop=(j == T - 1))
                # finalize both branches
                rg_t = wrk.tile([128, 128], F32, tag="rgt")
                for br, (tab, gb, rw, rb) in enumerate(((hcptab, gcb_s, rcw_s, rcb_s),
                                                       (htptab, gtb_s, rtw_s, rtb_s))):
                    dis_g = disr[:, g, br:br + 1]
                    t1 = wrk.tile([128, HD], F32, tag="t1f")
                    nc.vector.tensor_scalar(out=t1[:], in0=pg[:, br * HD:(br + 1) * HD],
                                            scalar1=dis_g, scalar2=None, op0=ALU.mult)
                    d2 = wrk.tile([128, 1], F32, tag="d2f")
                    nc.vector.tensor_tensor(out=d2[:], in0=dis_g, in1=dis_g, op=ALU.mult)
                    hp = wrk.tile([128, HD], F32, tag="hpl")
                    nc.sync.dma_start(hp[:], tab[g * 128:(g + 1) * 128, :])
                    t2 = wrk.tile([128, HD], F32, tag="t2f")
                    nc.vector.tensor_scalar(out=t2[:], in0=hp[:], scalar1=d2[:], scalar2=None, op0=ALU.mult)
                    nc.vector.tensor_tensor(out=t1[:], in0=t1[:], in1=t2[:], op=ALU.add)
                    nc.vector.tensor_tensor(out=t1[:], in0=t1[:], in1=gb[:], op=ALU.add)
                    # readout linear
                    gtp = ps.tile([128, 128], F32, tag="mmB")
                    nc.tensor.transpose(gtp[:HD, :], t1[:], ident[:])
                    gts = wrk.tile([HD, 128], F32, tag="gts")
                    nc.vector.tensor_copy(gts[:], gtp[:HD, :])
                    rp = ps.tile([128, HD], F32, tag="mmA")
                    nc.tensor.matmul(rp[:], lhsT=gts[:], rhs=rw[:], start=True, stop=True)
                    nc.vector.tensor_tensor(out=rg_t[:, br * HD:(br + 1) * HD], in0=rp[:], in1=rb[:], op=ALU.add)
                ohb = wrk.tile([128, 128], F32, tag="ohb")
                nc.vector.tensor_scalar(out=ohb[:], in0=iota[:], scalar1=bw_s[:, g:g + 1],
                                        scalar2=None, op0=ALU.is_equal)
                nc.tensor.matmul(pp[:], lhsT=ohb[:], rhs=rg_t[:], start=(g == 0), stop=(g == NG - 1))
                col0 += T

            pres = wrk.tile([128, 128], F32, tag="pres")
            nc.vector.tensor_copy(pres[:], pp[:])
            nc.sync.dma_start(pool_out[:, :], pres[:])

    nc.compile()
    return nc
